# revision 25
# baseline (speedup 1.0000x reference)
"""BiQRNN forward kernel for Trainium2 (8 NeuronCores, batch-sharded).

Model (see reference):
  ev  = X[:,:,0] (int ids), num = X[:,:,1:]
  e   = emb[ev]                      [B,S,256]
  n   = num @ Wn + bn                [B,S,4]
  c   = [e, n]                       [B,S,260]
  g   = c @ W + b  (W in {Wf,Wb})    -> Z = tanh(g[:,:512]), F = sigmoid(g[:,512:1024])
  hf  = fo_pool(Zf,Ff)[-1]  (h_t = F h_{t-1} + (1-F) Z)
  hb  = (1-Fb[S-1]) * Zb[S-1]        (only last step of reversed scan survives)
  out = [hf, hb] @ Wo + bo           [B,1]

Device strategy per core (8 batches each):
  - 4 indirect row-gathers per batch -> e_b [128, 4, 256] bf16 (gpsimd),
    software-pipelined one batch ahead of the compute stages
  - PE transposes (128x128 blocks) -> tp psum (bf16) -> eT_b [128, 2, 512]
    drained by the scalar engine
  - gate GEMM: G^T[h, tok] via matmul(lhsT=W-chunk, rhs=eT-slice), 2 K-passes
    + num/bias pass (K=8) in 2 concurrent PE quadrant row-groups
  - scalar engine: tanh/sigmoid straight from PSUM into chunked scan buffers
  - delta-form fo-pool: dz_t = z_{t-1} - z_t (ONE vector/gpsimd TT subtract),
    m_t = (dz_t + m_{t-1}) * s_t (one vector scan), h_T = m_T + z_T
    (avoids the more expensive (s-1)*z scalar_tensor_tensor form)
  - output projection via small accumulating matmuls (backward Wo pre-negated)
  - PE warmup stream keeps the clock-gate/p-state at full rate
"""
import numpy as np

import concourse.bacc as bacc
import concourse.bass as bass
import concourse.mybir as mybir
import concourse.tile as tile
from concourse import bass_utils

F32 = mybir.dt.float32
BF16 = mybir.dt.bfloat16
I32 = mybir.dt.int32
NP_BF16 = mybir.dt.np(BF16)

VOCAB, EMB, HID, OUT = 1000, 256, 512, 1
NUM_IN, NUM_OUT = 7, 4
B, S = 64, 512
NCORES = 8
BC = B // NCORES          # 8 batches per core
NT = BC * S               # 4096 tokens per core
SR = S + 1                # per-j-chunk segment (with separator column)
FLAT = 4 * SR             # 2052 flat scan columns
AF = mybir.ActivationFunctionType
ALU = mybir.AluOpType

# ---- tuning knobs ----
N_WARMUP_MM = 48
DRAIN_ENGINES = ["scalar"] * BC                    # eT psum->sbuf drain
DTT_ENGINES = ["vector"] * BC                      # dz = z_prev - z engine
XBAR_BATCHES = ()                        # batches using DMA-xbar eT


def build_kernel(debug=False):
    nc = bacc.Bacc("TRN2", target_bir_lowering=False, debug=debug)

    idx_d = nc.dram_tensor("idx32", [128, 4 * BC], I32, kind="ExternalInput")
    numt1_d = nc.dram_tensor("numt1", [128, NT], BF16, kind="ExternalInput")
    emb_d = nc.dram_tensor("emb", [VOCAB, EMB], BF16, kind="ExternalInput")
    wf_d = nc.dram_tensor("wf", [128, 2 * 2 * HID], BF16, kind="ExternalInput")
    wnfb_d = nc.dram_tensor("wnfb", [128, 2 * HID], BF16, kind="ExternalInput")
    wb_d = nc.dram_tensor("wb", [128, 2 * 2 * HID], BF16, kind="ExternalInput")
    wnbb_d = nc.dram_tensor("wnbb", [128, 2 * HID], BF16, kind="ExternalInput")
    wo_d = nc.dram_tensor("wo", [128, 8], F32, kind="ExternalInput")
    ident_d = nc.dram_tensor("ident", [128, 128], BF16, kind="ExternalInput")
    bo_d = nc.dram_tensor("bo", [1, 1], BF16, kind="ExternalInput")
    out_d = nc.dram_tensor("out", [BC, 1], F32, kind="ExternalOutput")

    with tile.TileContext(nc) as tc:
        with tc.tile_pool(name="const", bufs=1) as cpool, \
             tc.tile_pool(name="gath", bufs=6) as gpool, \
             tc.tile_pool(name="work", bufs=4) as wpool, \
             tc.tile_pool(name="ps", bufs=2, space="PSUM") as ps, \
             tc.tile_pool(name="pst", bufs=4, space="PSUM") as pst:
            # ---- constant loads ----
            idx_sb = cpool.tile([128, 4 * BC], I32)
            nc.sync.dma_start(out=idx_sb[:], in_=idx_d[:])
            wf_sb = cpool.tile([128, 2048], BF16)
            nc.sync.dma_start(out=wf_sb[:], in_=wf_d[:])
            wb_sb = cpool.tile([128, 2048], BF16)
            nc.sync.dma_start(out=wb_sb[:], in_=wb_d[:])
            wnfb_sb = cpool.tile([128, 1024], BF16)
            nc.sync.dma_start(out=wnfb_sb[:], in_=wnfb_d[:])
            wnbb_sb = cpool.tile([128, 1024], BF16)
            nc.sync.dma_start(out=wnbb_sb[:], in_=wnbb_d[:])
            numt1_sb = cpool.tile([128, NT], BF16)
            nc.sync.dma_start(out=numt1_sb[:], in_=numt1_d[:])
            wo_sb = cpool.tile([128, 8], F32)
            nc.sync.dma_start(out=wo_sb[:], in_=wo_d[:])
            bo_sb = cpool.tile([1, 1], BF16)
            nc.sync.dma_start(out=bo_sb[:], in_=bo_d[:])
            ident = cpool.tile([128, 128], BF16)
            nc.sync.dma_start(out=ident[:], in_=ident_d[:])

            # ---- PE warmup: keep p-state/clock-gate at full rate ----
            warm_src = cpool.tile([128, 256], BF16)
            nc.vector.memset(warm_src[:], 0.0)
            wps = ps.tile([128, 2, S], F32, tag="z")
            for i in range(N_WARMUP_MM):
                nc.tensor.matmul(wps[:, 0, 0:256], lhsT=warm_src[:, 0:128],
                                 rhs=warm_src[:], start=True, stop=True)
            # dummy reader: forces a WAR dependency so batch-0 gate matmuls
            # cannot race the warmup stream on the recycled PSUM bank
            warm_sink = cpool.tile([128, 256], BF16)
            nc.vector.tensor_copy(out=warm_sink[:], in_=wps[:, 0, 0:256])

            def eng(name):
                return {"vector": nc.vector, "gpsimd": nc.gpsimd}[name]

            def build_eT(b):
                """gather + transpose -> eT_b [128 d, 2 khalf, 512 tok] bf16"""
                eT_b = wpool.tile([128, 2, S], BF16, tag="eT")
                if b in XBAR_BATCHES:
                    e_b = gpool.tile([128, 4, EMB], BF16, tag="eg")
                    for g in range(4):
                        nc.gpsimd.indirect_dma_start(
                            out=e_b[:, g, :], out_offset=None, in_=emb_d[:],
                            in_offset=bass.IndirectOffsetOnAxis(
                                ap=idx_sb[:, b * 4 + g:b * 4 + g + 1], axis=0))
                    for k in range(2):
                        for g in range(4):
                            nc.sync.dma_start(
                                out=eT_b[:, k, g * 128:(g + 1) * 128],
                                in_=e_b[:, g, k * 128:(k + 1) * 128],
                                transpose=True)
                    return eT_b
                e_b = gpool.tile([128, 4, EMB], BF16, tag="eg")
                for g in range(4):
                    nc.gpsimd.indirect_dma_start(
                        out=e_b[:, g, :], out_offset=None, in_=emb_d[:],
                        in_offset=bass.IndirectOffsetOnAxis(
                            ap=idx_sb[:, b * 4 + g:b * 4 + g + 1], axis=0))
                tp = pst.tile([128, 8, 128], BF16, tag="f")
                for k in range(2):
                    for g in range(4):
                        nc.tensor.transpose(
                            out=tp[:, k * 4 + g, :],
                            in_=e_b[:, g, k * 128:(k + 1) * 128],
                            identity=ident[:])
                if DRAIN_ENGINES[b] == "scalar":
                    nc.scalar.copy(out=eT_b[:], in_=tp[:])
                else:
                    nc.vector.tensor_copy(out=eT_b[:], in_=tp[:])
                return eT_b

            def gate_mm12(out_ps, w_sb, col, rhs_e0, rhs_e1):
                nc.tensor.matmul(out_ps, lhsT=w_sb[:, col:col + 128],
                                 rhs=rhs_e0, start=True, stop=False)
                nc.tensor.matmul(out_ps, lhsT=w_sb[:, 1024 + col:1024 + col + 128],
                                 rhs=rhs_e1, start=False, stop=False)

            def gate_mm3p(out_ps, wn_sb, col, rhs_n, strip):
                kw = {}
                if strip > 0:
                    kw = dict(tile_position=(32 * strip, 0), skip_group_check=True)
                nc.tensor.matmul(out_ps,
                                 lhsT=wn_sb[32 * strip:32 * strip + NUM_IN + 1,
                                            col:col + 128],
                                 rhs=rhs_n[32 * strip:32 * strip + NUM_IN + 1, :],
                                 start=False, stop=True, **kw)

            # hS[h128, j, b]: forward final states; wtb: backward (s-1)*z
            hS = cpool.tile([128, 4, BC], F32)
            wtb = cpool.tile([128, 4, BC], F32)
            eTlast = cpool.tile([128, 2, BC], BF16)

            def bwd_stage():
                # backward direction: only t = S-1 matters
                rhs_e0 = eTlast[:, 0, :]
                rhs_e1 = eTlast[:, 1, :]
                rhs_n = numt1_sb[:, S - 1::S]     # [8-strips, BC]
                zbps = ps.tile([128, 4, BC], F32, tag="z")
                fbps = ps.tile([128, 4, BC], F32, tag="z")
                for j in range(4):
                    gate_mm12(zbps[:, j, :], wb_sb, j * 128, rhs_e0, rhs_e1)
                    gate_mm3p(zbps[:, j, :], wnbb_sb, j * 128, rhs_n, 0)
                for j in range(4):
                    gate_mm12(fbps[:, j, :], wb_sb, 512 + j * 128, rhs_e0, rhs_e1)
                    gate_mm3p(fbps[:, j, :], wnbb_sb, 512 + j * 128, rhs_n, 0)
                zb_t = wpool.tile([128, 4, BC], F32, tag="zb")
                sb_t = wpool.tile([128, 4, BC], F32, tag="sb")
                nc.scalar.activation(zb_t[:], zbps[:], AF.Tanh)
                nc.scalar.activation(sb_t[:], fbps[:], AF.Sigmoid)
                nc.vector.scalar_tensor_tensor(
                    out=wtb[:], in0=sb_t[:], scalar=1.0, in1=zb_t[:],
                    op0=ALU.subtract, op1=ALU.mult)

            # ---- forward pipeline, per batch (software-pipelined eT) ----
            eTs = {0: build_eT(0)}
            for b in range(BC):
                tok = slice(b * S, (b + 1) * S)
                if b + 1 < BC:
                    eTs[b + 1] = build_eT(b + 1)
                eT_b = eTs.pop(b)
                nc.vector.tensor_copy(out=eTlast[:, :, b], in_=eT_b[:, :, S - 1])
                rhs_e0 = eT_b[:, 0, :]
                rhs_e1 = eT_b[:, 1, :]
                rhs_n = numt1_sb[:, tok]
                z01 = ps.tile([128, 2, S], F32, tag="z")
                z23 = ps.tile([128, 2, S], F32, tag="z")
                f0 = pst.tile([128, S], F32, tag="f")
                f1 = pst.tile([128, S], F32, tag="f")
                f2 = pst.tile([128, S], F32, tag="f")
                f3 = pst.tile([128, S], F32, tag="f")
                ztiles = [z01[:, 0, :], z01[:, 1, :], z23[:, 0, :], z23[:, 1, :]]
                ftiles = [f0[:], f1[:], f2[:], f3[:]]
                for j in range(4):
                    gate_mm12(ztiles[j], wf_sb, j * 128, rhs_e0, rhs_e1)
                for j in range(4):
                    gate_mm3p(ztiles[j], wnfb_sb, j * 128, rhs_n, j % 2)
                for j in range(4):
                    gate_mm12(ftiles[j], wf_sb, 512 + j * 128, rhs_e0, rhs_e1)
                for j in range(4):
                    gate_mm3p(ftiles[j], wnfb_sb, 512 + j * 128, rhs_n, j % 2)

                # chunk-local scan buffers:
                #   z_buf[:, j, 0] = 0 (zero lead); [:, j, 1+t] = z(j,t)
                #   s_buf[:, j, t] = s(j,t); [:, j, 512] = 0 (separator)
                z_buf = wpool.tile([128, 4, SR], BF16, tag="z")
                s_buf = wpool.tile([128, 4, SR], BF16, tag="s")
                nc.vector.memset(z_buf[:, :, 0], 0.0)
                nc.vector.memset(s_buf[:, :, S], 0.0)
                nc.scalar.activation(z_buf[:, 0:2, 1:SR], z01[:], AF.Tanh)
                nc.scalar.activation(z_buf[:, 2:4, 1:SR], z23[:], AF.Tanh)
                for j in range(4):
                    nc.scalar.activation(s_buf[:, j, 0:S], ftiles[j], AF.Sigmoid)
                # dz(j,t) = z(j,t-1) - z(j,t); col 512 garbage (s=0 resets m)
                dz = wpool.tile([128, 4, SR], BF16, tag="dz")
                eng(DTT_ENGINES[b]).tensor_tensor(
                    out=dz[:, :, 0:S], in0=z_buf[:, :, 0:S], in1=z_buf[:, :, 1:SR],
                    op=ALU.subtract)
                # m_t = (dz_t + m_{t-1}) * s_t ; separators (s=0) reset m
                m_buf = wpool.tile([128, 4, SR], BF16, tag="m")
                nc.vector.tensor_tensor_scan(
                    out=m_buf[:].opt(), data0=dz[:].opt(), data1=s_buf[:].opt(),
                    initial=0.0, op0=ALU.add, op1=ALU.mult)
                # h_final(j) = m(j, 511) + z(j, 511)
                nc.vector.tensor_tensor(
                    out=hS[:, :, b],
                    in0=m_buf[:, :, S - 1],
                    in1=z_buf[:, :, S],
                    op=ALU.add)

            bwd_stage()

            # ---- output projection ----
            # out[b] = sum_j hS[:,j,b].Wo_j - wtb[:,j,b].Wo_bj + bo
            # (wo columns 4..7 hold NEGATED backward Wo chunks)
            ops = pst.tile([BC, 1], F32, tag="f")
            for j in range(4):
                nc.tensor.matmul(ops[:], lhsT=hS[:, j, :], rhs=wo_sb[:, j:j + 1],
                                 start=(j == 0), stop=False)
            for j in range(4):
                nc.tensor.matmul(ops[:], lhsT=wtb[:, j, :], rhs=wo_sb[:, 4 + j:5 + j],
                                 start=False, stop=False)
            ones_sb = cpool.tile([1, BC], BF16)
            nc.vector.memset(ones_sb[:], 1.0)
            nc.tensor.matmul(ops[:], lhsT=ones_sb[:],
                             rhs=bo_sb[:], start=False, stop=True)
            out_sb = cpool.tile([BC, 1], F32)
            nc.vector.tensor_copy(out=out_sb[:], in_=ops[:])
            nc.sync.dma_start(out=out_d[:], in_=out_sb[:])

    nc.compile()
    return nc


def prep_inputs(X, emb, Wn, bn, Wf, bf, Wb, bb, Wo, bo):
    """Host-side sharding + weight folding. Returns per-core input maps."""
    X = np.asarray(X, np.float32)
    emb = np.asarray(emb, np.float32)
    Wn = np.asarray(Wn, np.float32)
    bn = np.asarray(bn, np.float32)
    Wf = np.asarray(Wf, np.float32)
    bf_ = np.asarray(bf, np.float32)
    Wb = np.asarray(Wb, np.float32)
    bb_ = np.asarray(bb, np.float32)
    Wo = np.asarray(Wo, np.float32)
    bo_ = np.asarray(bo, np.float32)

    ev = X[:, :, 0].astype(np.int32)                       # [B,S]
    num = X[:, :, 1:]                                      # [B,S,7]

    def fold(W, bvec):
        Wzf = W[:, :2 * HID]                               # drop unused O gate
        w_emb = Wzf[:EMB]                                  # [256,1024]
        wf_resh = w_emb.reshape(2, 128, 2 * HID).transpose(1, 0, 2).reshape(128, 2 * 2 * HID)
        wnf = Wn @ Wzf[EMB:]                               # [7,1024]
        bias_eff = bvec[:2 * HID] + bn @ Wzf[EMB:]         # [1024]
        wnfb = np.concatenate([wnf, bias_eff[None, :]], axis=0)  # [8,1024]
        wnfb_rep = np.zeros((128, 2 * HID), np.float32)
        for strip in range(4):
            wnfb_rep[32 * strip:32 * strip + NUM_IN + 1] = wnfb
        return wf_resh.astype(NP_BF16), wnfb_rep.astype(NP_BF16)

    wf_resh, wnfb = fold(Wf, bf_)
    wb_resh, wnbb = fold(Wb, bb_)

    wo_resh = np.empty((128, 8), np.float32)
    for j in range(4):
        wo_resh[:, j] = Wo[j * 128:(j + 1) * 128, 0]
        wo_resh[:, 4 + j] = -Wo[HID + j * 128:HID + (j + 1) * 128, 0]

    emb_bf = emb.astype(NP_BF16)
    bo_bf = bo_.reshape(1, 1).astype(NP_BF16)

    in_maps = []
    for c in range(NCORES):
        bs = slice(c * BC, (c + 1) * BC)
        # token t = g*128 + p of local batch b sits at idx32[p, b*4 + g]
        ev_core = ev[bs]                                    # [BC, S]
        idx_wrapped = np.ascontiguousarray(
            ev_core.reshape(BC, 4, 128).transpose(2, 0, 1).reshape(128, 4 * BC))
        numt = num[bs].transpose(2, 0, 1).reshape(NUM_IN, NT)
        numt1 = np.zeros((128, NT), np.float32)
        for strip in range(4):
            numt1[32 * strip:32 * strip + NUM_IN] = numt
            numt1[32 * strip + NUM_IN] = 1.0
        numt1 = numt1.astype(NP_BF16)
        in_maps.append({
            "idx32": idx_wrapped,
            "ident": np.eye(128, dtype=np.float32).astype(NP_BF16),
            "numt1": numt1,
            "emb": emb_bf,
            "wf": wf_resh, "wnfb": wnfb,
            "wb": wb_resh, "wnbb": wnbb,
            "wo": wo_resh, "bo": bo_bf,
        })
    return in_maps


_NC_CACHE = {}


def kernel(X, emb, Wn, bn, Wf, bf, Wb, bb, Wo, bo):
    if "nc" not in _NC_CACHE:
        _NC_CACHE["nc"] = build_kernel()
    nc = _NC_CACHE["nc"]
    in_maps = prep_inputs(X, emb, Wn, bn, Wf, bf, Wb, bb, Wo, bo)
    res = bass_utils.run_bass_kernel_spmd(nc, in_maps, core_ids=list(range(NCORES)))
    return np.concatenate([res.results[c]["out"] for c in range(NCORES)], axis=0)


# revision 26
# speedup vs baseline: 1.0103x; 1.0103x over previous
"""BiQRNN forward kernel for Trainium2 (8 NeuronCores, batch-sharded).

Model (see reference):
  ev  = X[:,:,0] (int ids), num = X[:,:,1:]
  e   = emb[ev]                      [B,S,256]
  n   = num @ Wn + bn                [B,S,4]
  c   = [e, n]                       [B,S,260]
  g   = c @ W + b  (W in {Wf,Wb})    -> Z = tanh(g[:,:512]), F = sigmoid(g[:,512:1024])
  hf  = fo_pool(Zf,Ff)[-1]  (h_t = F h_{t-1} + (1-F) Z)
  hb  = (1-Fb[S-1]) * Zb[S-1]        (only last step of reversed scan survives)
  out = [hf, hb] @ Wo + bo           [B,1]

Device strategy per core (8 batches each):
  - 4 indirect row-gathers per batch -> e_b [128, 4, 256] bf16 (gpsimd),
    software-pipelined one batch ahead of the compute stages
  - PE transposes (128x128 blocks) -> tp psum (bf16) -> eT_b [128, 2, 512]
    drained by the scalar engine
  - gate GEMM: G^T[h, tok] via matmul(lhsT=W-chunk, rhs=eT-slice), 2 K-passes
    + num/bias pass (K=8) in 2 concurrent PE quadrant row-groups
  - scalar engine: tanh/sigmoid straight from PSUM into chunked scan buffers
  - delta-form fo-pool: dz_t = z_{t-1} - z_t (ONE vector/gpsimd TT subtract),
    m_t = (dz_t + m_{t-1}) * s_t (one vector scan), h_T = m_T + z_T
    (avoids the more expensive (s-1)*z scalar_tensor_tensor form)
  - output projection via small accumulating matmuls (backward Wo pre-negated)
  - PE warmup stream keeps the clock-gate/p-state at full rate
"""
import numpy as np

import concourse.bacc as bacc
import concourse.bass as bass
import concourse.mybir as mybir
import concourse.tile as tile
from concourse import bass_utils

F32 = mybir.dt.float32
BF16 = mybir.dt.bfloat16
I32 = mybir.dt.int32
NP_BF16 = mybir.dt.np(BF16)

VOCAB, EMB, HID, OUT = 1000, 256, 512, 1
NUM_IN, NUM_OUT = 7, 4
B, S = 64, 512
NCORES = 8
BC = B // NCORES          # 8 batches per core
NT = BC * S               # 4096 tokens per core
SR = S + 1                # per-j-chunk segment (with separator column)
FLAT = 4 * SR             # 2052 flat scan columns
AF = mybir.ActivationFunctionType
ALU = mybir.AluOpType

# ---- tuning knobs ----
N_WARMUP_MM = 48
DRAIN_ENGINES = ["scalar"] * BC                    # eT psum->sbuf drain
DTT_ENGINES = ["vector"] * BC                      # dz = z_prev - z engine
XBAR_BATCHES = ()                        # batches using DMA-xbar eT


def build_kernel(debug=False):
    nc = bacc.Bacc("TRN2", target_bir_lowering=False, debug=debug)

    idx_d = nc.dram_tensor("idx32", [128, 4 * BC], I32, kind="ExternalInput")
    numt1_d = nc.dram_tensor("numt1", [128, NT], BF16, kind="ExternalInput")
    emb_d = nc.dram_tensor("emb", [VOCAB, EMB], BF16, kind="ExternalInput")
    wf_d = nc.dram_tensor("wf", [128, 2 * 2 * HID], BF16, kind="ExternalInput")
    wnfb_d = nc.dram_tensor("wnfb", [128, 2 * HID], BF16, kind="ExternalInput")
    wb_d = nc.dram_tensor("wb", [128, 2 * 2 * HID], BF16, kind="ExternalInput")
    wnbb_d = nc.dram_tensor("wnbb", [128, 2 * HID], BF16, kind="ExternalInput")
    wo_d = nc.dram_tensor("wo", [128, 8], F32, kind="ExternalInput")
    ident_d = nc.dram_tensor("ident", [128, 128], BF16, kind="ExternalInput")
    bo_d = nc.dram_tensor("bo", [1, 1], BF16, kind="ExternalInput")
    out_d = nc.dram_tensor("out", [BC, 1], F32, kind="ExternalOutput")

    with tile.TileContext(nc) as tc:
        with tc.tile_pool(name="const", bufs=1) as cpool, \
             tc.tile_pool(name="gath", bufs=6) as gpool, \
             tc.tile_pool(name="work", bufs=3) as wpool, \
             tc.tile_pool(name="ps", bufs=2, space="PSUM") as ps, \
             tc.tile_pool(name="pst", bufs=4, space="PSUM") as pst:
            # ---- constant loads ----
            idx_sb = cpool.tile([128, 4 * BC], I32)
            nc.sync.dma_start(out=idx_sb[:], in_=idx_d[:])
            wf_sb = cpool.tile([128, 2048], BF16)
            nc.sync.dma_start(out=wf_sb[:], in_=wf_d[:])
            wb_sb = cpool.tile([128, 2048], BF16)
            nc.sync.dma_start(out=wb_sb[:], in_=wb_d[:])
            wnfb_sb = cpool.tile([128, 1024], BF16)
            nc.sync.dma_start(out=wnfb_sb[:], in_=wnfb_d[:])
            wnbb_sb = cpool.tile([128, 1024], BF16)
            nc.sync.dma_start(out=wnbb_sb[:], in_=wnbb_d[:])
            numt1_sb = cpool.tile([128, NT], BF16)
            nc.sync.dma_start(out=numt1_sb[:], in_=numt1_d[:])
            wo_sb = cpool.tile([128, 8], F32)
            nc.sync.dma_start(out=wo_sb[:], in_=wo_d[:])
            bo_sb = cpool.tile([1, 1], BF16)
            nc.sync.dma_start(out=bo_sb[:], in_=bo_d[:])
            ident = cpool.tile([128, 128], BF16)
            nc.sync.dma_start(out=ident[:], in_=ident_d[:])

            # ---- PE warmup: keep p-state/clock-gate at full rate ----
            warm_src = cpool.tile([128, 256], BF16)
            nc.vector.memset(warm_src[:], 0.0)
            wps = ps.tile([128, 2, S], F32, tag="z")
            for i in range(N_WARMUP_MM):
                nc.tensor.matmul(wps[:, 0, 0:256], lhsT=warm_src[:, 0:128],
                                 rhs=warm_src[:], start=True, stop=True)
            # dummy reader: forces a WAR dependency so batch-0 gate matmuls
            # cannot race the warmup stream on the recycled PSUM bank
            warm_sink = cpool.tile([128, 256], BF16)
            nc.vector.tensor_copy(out=warm_sink[:], in_=wps[:, 0, 0:256])

            def eng(name):
                return {"vector": nc.vector, "gpsimd": nc.gpsimd}[name]

            def build_eT(b):
                """gather + transpose -> eT_b [128 d, 2 khalf, 512 tok] bf16"""
                eT_b = wpool.tile([128, 2, S], BF16, tag="eT")
                if b in XBAR_BATCHES:
                    e_b = gpool.tile([128, 4, EMB], BF16, tag="eg")
                    for g in range(4):
                        nc.gpsimd.indirect_dma_start(
                            out=e_b[:, g, :], out_offset=None, in_=emb_d[:],
                            in_offset=bass.IndirectOffsetOnAxis(
                                ap=idx_sb[:, b * 4 + g:b * 4 + g + 1], axis=0))
                    for k in range(2):
                        for g in range(4):
                            nc.sync.dma_start(
                                out=eT_b[:, k, g * 128:(g + 1) * 128],
                                in_=e_b[:, g, k * 128:(k + 1) * 128],
                                transpose=True)
                    return eT_b
                e_b = gpool.tile([128, 4, EMB], BF16, tag="eg")
                for g in range(4):
                    nc.gpsimd.indirect_dma_start(
                        out=e_b[:, g, :], out_offset=None, in_=emb_d[:],
                        in_offset=bass.IndirectOffsetOnAxis(
                            ap=idx_sb[:, b * 4 + g:b * 4 + g + 1], axis=0))
                tp = pst.tile([128, 8, 128], BF16, tag="f")
                for k in range(2):
                    for g in range(4):
                        nc.tensor.transpose(
                            out=tp[:, k * 4 + g, :],
                            in_=e_b[:, g, k * 128:(k + 1) * 128],
                            identity=ident[:])
                if DRAIN_ENGINES[b] == "scalar":
                    nc.scalar.copy(out=eT_b[:], in_=tp[:])
                else:
                    nc.vector.tensor_copy(out=eT_b[:], in_=tp[:])
                return eT_b

            def gate_mm12(out_ps, w_sb, col, rhs_e0, rhs_e1):
                nc.tensor.matmul(out_ps, lhsT=w_sb[:, col:col + 128],
                                 rhs=rhs_e0, start=True, stop=False)
                nc.tensor.matmul(out_ps, lhsT=w_sb[:, 1024 + col:1024 + col + 128],
                                 rhs=rhs_e1, start=False, stop=False)

            def gate_mm3p(out_ps, wn_sb, col, rhs_n, strip):
                kw = {}
                if strip > 0:
                    kw = dict(tile_position=(32 * strip, 0), skip_group_check=True)
                nc.tensor.matmul(out_ps,
                                 lhsT=wn_sb[32 * strip:32 * strip + NUM_IN + 1,
                                            col:col + 128],
                                 rhs=rhs_n[32 * strip:32 * strip + NUM_IN + 1, :],
                                 start=False, stop=True, **kw)

            # hS[h128, j, b]: forward final states; wtb: backward (s-1)*z
            hS = cpool.tile([128, 4, BC], F32)
            wtb = cpool.tile([128, 4, BC], F32)
            eTlast = cpool.tile([128, 2, BC], BF16)

            def bwd_stage():
                # backward direction: only t = S-1 matters
                rhs_e0 = eTlast[:, 0, :]
                rhs_e1 = eTlast[:, 1, :]
                rhs_n = numt1_sb[:, S - 1::S]     # [8-strips, BC]
                zbps = ps.tile([128, 4, BC], F32, tag="z")
                fbps = ps.tile([128, 4, BC], F32, tag="z")
                for j in range(4):
                    gate_mm12(zbps[:, j, :], wb_sb, j * 128, rhs_e0, rhs_e1)
                    gate_mm3p(zbps[:, j, :], wnbb_sb, j * 128, rhs_n, 0)
                for j in range(4):
                    gate_mm12(fbps[:, j, :], wb_sb, 512 + j * 128, rhs_e0, rhs_e1)
                    gate_mm3p(fbps[:, j, :], wnbb_sb, 512 + j * 128, rhs_n, 0)
                zb_t = wpool.tile([128, 4, BC], F32, tag="zb")
                sb_t = wpool.tile([128, 4, BC], F32, tag="sb")
                nc.scalar.activation(zb_t[:], zbps[:], AF.Tanh)
                nc.scalar.activation(sb_t[:], fbps[:], AF.Sigmoid)
                nc.vector.scalar_tensor_tensor(
                    out=wtb[:], in0=sb_t[:], scalar=1.0, in1=zb_t[:],
                    op0=ALU.subtract, op1=ALU.mult)

            # ---- forward pipeline, per batch (software-pipelined eT) ----
            eTs = {0: build_eT(0)}
            for b in range(BC):
                tok = slice(b * S, (b + 1) * S)
                if b + 1 < BC:
                    eTs[b + 1] = build_eT(b + 1)
                eT_b = eTs.pop(b)
                nc.vector.tensor_copy(out=eTlast[:, :, b], in_=eT_b[:, :, S - 1])
                rhs_e0 = eT_b[:, 0, :]
                rhs_e1 = eT_b[:, 1, :]
                rhs_n = numt1_sb[:, tok]
                z01 = ps.tile([128, 2, S], F32, tag="z")
                z23 = ps.tile([128, 2, S], F32, tag="z")
                f0 = pst.tile([128, S], F32, tag="f")
                f1 = pst.tile([128, S], F32, tag="f")
                f2 = pst.tile([128, S], F32, tag="f")
                f3 = pst.tile([128, S], F32, tag="f")
                ztiles = [z01[:, 0, :], z01[:, 1, :], z23[:, 0, :], z23[:, 1, :]]
                ftiles = [f0[:], f1[:], f2[:], f3[:]]
                for j in range(4):
                    gate_mm12(ztiles[j], wf_sb, j * 128, rhs_e0, rhs_e1)
                for j in range(4):
                    gate_mm3p(ztiles[j], wnfb_sb, j * 128, rhs_n, j % 2)
                for j in range(4):
                    gate_mm12(ftiles[j], wf_sb, 512 + j * 128, rhs_e0, rhs_e1)
                for j in range(4):
                    gate_mm3p(ftiles[j], wnfb_sb, 512 + j * 128, rhs_n, j % 2)

                # chunk-local scan buffers:
                #   z_buf[:, j, 0] = 0 (zero lead); [:, j, 1+t] = z(j,t)
                #   s_buf[:, j, t] = s(j,t); [:, j, 512] = 0 (separator)
                z_buf = wpool.tile([128, 4, SR], BF16, tag="z")
                s_buf = wpool.tile([128, 4, SR], BF16, tag="s")
                nc.vector.memset(z_buf[:, :, 0], 0.0)
                nc.vector.memset(s_buf[:, :, S], 0.0)
                nc.scalar.activation(z_buf[:, 0:2, 1:SR], z01[:], AF.Tanh)
                nc.scalar.activation(z_buf[:, 2:4, 1:SR], z23[:], AF.Tanh)
                for j in range(4):
                    nc.scalar.activation(s_buf[:, j, 0:S], ftiles[j], AF.Sigmoid)
                # dz(j,t) = z(j,t-1) - z(j,t); col 512 garbage (s=0 resets m)
                dz = wpool.tile([128, 4, SR], BF16, tag="dz")
                eng(DTT_ENGINES[b]).tensor_tensor(
                    out=dz[:, :, 0:S], in0=z_buf[:, :, 0:S], in1=z_buf[:, :, 1:SR],
                    op=ALU.subtract)
                # m_t = (dz_t + m_{t-1}) * s_t ; separators (s=0) reset m
                m_buf = wpool.tile([128, 4, SR], BF16, tag="m")
                nc.vector.tensor_tensor_scan(
                    out=m_buf[:].opt(), data0=dz[:].opt(), data1=s_buf[:].opt(),
                    initial=0.0, op0=ALU.add, op1=ALU.mult)
                # h_final(j) = m(j, 511) + z(j, 511)
                nc.vector.tensor_tensor(
                    out=hS[:, :, b],
                    in0=m_buf[:, :, S - 1],
                    in1=z_buf[:, :, S],
                    op=ALU.add)

            bwd_stage()

            # ---- output projection ----
            # out[b] = sum_j hS[:,j,b].Wo_j - wtb[:,j,b].Wo_bj + bo
            # (wo columns 4..7 hold NEGATED backward Wo chunks)
            ops = pst.tile([BC, 1], F32, tag="f")
            for j in range(4):
                nc.tensor.matmul(ops[:], lhsT=hS[:, j, :], rhs=wo_sb[:, j:j + 1],
                                 start=(j == 0), stop=False)
            for j in range(4):
                nc.tensor.matmul(ops[:], lhsT=wtb[:, j, :], rhs=wo_sb[:, 4 + j:5 + j],
                                 start=False, stop=False)
            ones_sb = cpool.tile([1, BC], BF16)
            nc.vector.memset(ones_sb[:], 1.0)
            nc.tensor.matmul(ops[:], lhsT=ones_sb[:],
                             rhs=bo_sb[:], start=False, stop=True)
            out_sb = cpool.tile([BC, 1], F32)
            nc.vector.tensor_copy(out=out_sb[:], in_=ops[:])
            nc.sync.dma_start(out=out_d[:], in_=out_sb[:])

    nc.compile()
    return nc


def prep_inputs(X, emb, Wn, bn, Wf, bf, Wb, bb, Wo, bo):
    """Host-side sharding + weight folding. Returns per-core input maps."""
    X = np.asarray(X, np.float32)
    emb = np.asarray(emb, np.float32)
    Wn = np.asarray(Wn, np.float32)
    bn = np.asarray(bn, np.float32)
    Wf = np.asarray(Wf, np.float32)
    bf_ = np.asarray(bf, np.float32)
    Wb = np.asarray(Wb, np.float32)
    bb_ = np.asarray(bb, np.float32)
    Wo = np.asarray(Wo, np.float32)
    bo_ = np.asarray(bo, np.float32)

    ev = X[:, :, 0].astype(np.int32)                       # [B,S]
    num = X[:, :, 1:]                                      # [B,S,7]

    def fold(W, bvec):
        Wzf = W[:, :2 * HID]                               # drop unused O gate
        w_emb = Wzf[:EMB]                                  # [256,1024]
        wf_resh = w_emb.reshape(2, 128, 2 * HID).transpose(1, 0, 2).reshape(128, 2 * 2 * HID)
        wnf = Wn @ Wzf[EMB:]                               # [7,1024]
        bias_eff = bvec[:2 * HID] + bn @ Wzf[EMB:]         # [1024]
        wnfb = np.concatenate([wnf, bias_eff[None, :]], axis=0)  # [8,1024]
        wnfb_rep = np.zeros((128, 2 * HID), np.float32)
        for strip in range(4):
            wnfb_rep[32 * strip:32 * strip + NUM_IN + 1] = wnfb
        return wf_resh.astype(NP_BF16), wnfb_rep.astype(NP_BF16)

    wf_resh, wnfb = fold(Wf, bf_)
    wb_resh, wnbb = fold(Wb, bb_)

    wo_resh = np.empty((128, 8), np.float32)
    for j in range(4):
        wo_resh[:, j] = Wo[j * 128:(j + 1) * 128, 0]
        wo_resh[:, 4 + j] = -Wo[HID + j * 128:HID + (j + 1) * 128, 0]

    emb_bf = emb.astype(NP_BF16)
    bo_bf = bo_.reshape(1, 1).astype(NP_BF16)

    in_maps = []
    for c in range(NCORES):
        bs = slice(c * BC, (c + 1) * BC)
        # token t = g*128 + p of local batch b sits at idx32[p, b*4 + g]
        ev_core = ev[bs]                                    # [BC, S]
        idx_wrapped = np.ascontiguousarray(
            ev_core.reshape(BC, 4, 128).transpose(2, 0, 1).reshape(128, 4 * BC))
        numt = num[bs].transpose(2, 0, 1).reshape(NUM_IN, NT)
        numt1 = np.zeros((128, NT), np.float32)
        for strip in range(4):
            numt1[32 * strip:32 * strip + NUM_IN] = numt
            numt1[32 * strip + NUM_IN] = 1.0
        numt1 = numt1.astype(NP_BF16)
        in_maps.append({
            "idx32": idx_wrapped,
            "ident": np.eye(128, dtype=np.float32).astype(NP_BF16),
            "numt1": numt1,
            "emb": emb_bf,
            "wf": wf_resh, "wnfb": wnfb,
            "wb": wb_resh, "wnbb": wnbb,
            "wo": wo_resh, "bo": bo_bf,
        })
    return in_maps


_NC_CACHE = {}


def kernel(X, emb, Wn, bn, Wf, bf, Wb, bb, Wo, bo):
    if "nc" not in _NC_CACHE:
        _NC_CACHE["nc"] = build_kernel()
    nc = _NC_CACHE["nc"]
    in_maps = prep_inputs(X, emb, Wn, bn, Wf, bf, Wb, bb, Wo, bo)
    res = bass_utils.run_bass_kernel_spmd(nc, in_maps, core_ids=list(range(NCORES)))
    return np.concatenate([res.results[c]["out"] for c in range(NCORES)], axis=0)
